# revision 12
# baseline (speedup 1.0000x reference)
"""Llama4-style MoE (top-1 router + 8 GLU experts + shared GLU expert) on 8
Trainium2 NeuronCores.

Strategy (expert-parallel): the router is evaluated on the host as part of
sharding; tokens are gathered per expert and core e processes expert e's
tokens through its expert GLU, plus a fixed 1/8 shard of all tokens through
the (replicated) shared expert GLU. Matmuls run in bf16 with fp32 PSUM
accumulation. Outputs are scattered back and summed on the host.

Shapes are hardcoded for B=4, S=2048, H=I=2048, E=8.
"""

import sys

for _p in ("/opt/trn_rl_repo", "/root/.axon_site/_ro/trn_rl_repo"):
    if _p not in sys.path:
        sys.path.append(_p)

import numpy as np
import ml_dtypes

import concourse.bass as bass
import concourse.mybir as mybir
import concourse.tile as tile
from concourse import bacc
from concourse.bass_utils import run_bass_kernel_spmd

BF16 = ml_dtypes.bfloat16

P = 128
H = 2048
I = 2048
E = 8
T_TOTAL = 8192
KT = H // P  # 16 k-tiles
MT = I // P  # 16 m-tiles

CE = 1088  # per-core expert-token capacity (mean 1024; actual max for the
# fixed seed is 1078; the multi-pass fallback in kernel() covers overflow)
CS = T_TOTAL // E  # shared-expert tokens per core

EXP_BLOCKS = [(0, 512), (512, 512), (1024, 64)]
SH_BLOCKS = [(0, 512), (512, 512)]

_NC = None  # compiled Bass module (built once per process)
_WEIGHT_CACHE = {}  # id(array) -> preprocessed per-core weight shards
_RUNNER = None  # cached jitted SPMD executable (built once per process)
_DEV_WEIGHTS = {}  # weight id key -> device-resident concatenated shards

WEIGHT_NAMES = ("wg_e", "wu_e", "wd_e", "wg_s", "wu_s", "wd_s")
ACT_NAMES = ("xe", "xs", "sce", "scs")


def _build_nc(reps=1):
    dt = mybir.dt
    nc = bacc.Bacc("TRN2", target_bir_lowering=False, debug=False, num_devices=8)

    xe = nc.dram_tensor("xe", [P, KT, CE], dt.bfloat16, kind="ExternalInput").ap()
    xs = nc.dram_tensor("xs", [P, KT, CS], dt.bfloat16, kind="ExternalInput").ap()
    sce = nc.dram_tensor("sce", [P, CE], dt.float32, kind="ExternalInput").ap()
    scs = nc.dram_tensor("scs", [P, CS], dt.float32, kind="ExternalInput").ap()
    wts = {}
    for name in ("wg_e", "wu_e", "wd_e", "wg_s", "wu_s", "wd_s"):
        wts[name] = nc.dram_tensor(
            name, [P, MT, KT, P], dt.bfloat16, kind="ExternalInput"
        ).ap()
    ye = nc.dram_tensor("ye", [MT, P, CE], dt.bfloat16, kind="ExternalOutput").ap()
    ys = nc.dram_tensor("ys", [MT, P, CS], dt.bfloat16, kind="ExternalOutput").ap()

    with tile.TileContext(nc) as tc:
        with (
            tc.tile_pool(name="xpool", bufs=1) as xpool,
            tc.tile_pool(name="wpool", bufs=4) as wpool,
            tc.tile_pool(name="apool", bufs=1) as apool,
            tc.tile_pool(name="ypool", bufs=4) as ypool,
            tc.tile_pool(name="psum", bufs=2, space="PSUM") as psum,
        ):
            xe_sb = xpool.tile([P, KT, CE], dt.bfloat16, tag="xe")
            xs_sb = xpool.tile([P, KT, CS], dt.bfloat16, tag="xs")
            # startup critical path: first gate-weight tiles and xe[k=0] go
            # out before the bulk x loads so PE can start ~2 us in; per-k
            # xe loads + byte-range dep tracking let pass A's matmuls chase
            # the DMA stream
            pre_w = []
            for m in range(2):
                w_sb = wpool.tile([P, KT, P], dt.bfloat16, tag="w")
                pre_w.append(w_sb)
            nc.sync.dma_start(pre_w[0][:], wts["wg_e"][:, 0])
            nc.sync.dma_start(xe_sb[:, 0], xe[:, 0])
            nc.sync.dma_start(pre_w[1][:], wts["wg_e"][:, 1])
            for k in range(1, KT):
                nc.sync.dma_start(xe_sb[:, k], xe[:, k])
            sce_sb = xpool.tile([P, CE], dt.float32, tag="sce")
            scs_sb = xpool.tile([P, CS], dt.float32, tag="scs")
            ae_sb = apool.tile([P, MT, CE], dt.bfloat16, tag="ae")
            as_sb = apool.tile([P, MT, CS], dt.bfloat16, tag="as")

            groups = [
                (xe_sb, sce_sb, ae_sb, "wg_e", "wu_e", "wd_e", ye, EXP_BLOCKS),
                (xs_sb, scs_sb, as_sb, "wg_s", "wu_s", "wd_s", ys, SH_BLOCKS),
            ] * reps
            for gi, (x_sb, sc_sb, a_sb, wg_n, wu_n, wd_n, y_d, blocks) in enumerate(
                groups
            ):
                # ---- pass A: a = silu(Wg^T x) ----
                for m in range(MT):
                    if gi == 0 and m < 2:
                        w_sb = pre_w[m]
                    else:
                        w_sb = wpool.tile([P, KT, P], dt.bfloat16, tag="w")
                        nc.sync.dma_start(w_sb[:], wts[wg_n][:, m])
                    ps = [
                        psum.tile([P, 512], dt.float32, tag=f"ps{ti}", name=f"ps{ti}")
                        for ti in range(len(blocks))
                    ]
                    for k in range(KT):
                        lhs = w_sb[:, k, :]
                        for ti, (off, bl) in enumerate(blocks):
                            nc.tensor.matmul(
                                ps[ti][:, :bl],
                                lhs,
                                x_sb[:, k, off : off + bl],
                                start=(k == 0),
                                stop=(k == KT - 1),
                            )
                    for ti, (off, bl) in enumerate(blocks):
                        nc.scalar.activation(
                            a_sb[:, m, off : off + bl],
                            ps[ti][:, :bl],
                            mybir.ActivationFunctionType.Silu,
                        )
                if gi == 0:
                    # xs/sce/scs are first read in expert pass C and the
                    # shared group; issuing after pass A keeps them off the
                    # startup DMA critical path
                    for k in range(KT):
                        nc.sync.dma_start(xs_sb[:, k], xs[:, k])
                    nc.sync.dma_start(sce_sb[:], sce[:])
                    nc.sync.dma_start(scs_sb[:], scs[:])
                # ---- pass B: a *= Wu^T x ----
                for m in range(MT):
                    w_sb = wpool.tile([P, KT, P], dt.bfloat16, tag="w")
                    nc.sync.dma_start(w_sb[:], wts[wu_n][:, m])
                    ps = [
                        psum.tile([P, 512], dt.float32, tag=f"ps{ti}", name=f"ps{ti}")
                        for ti in range(len(blocks))
                    ]
                    for k in range(KT):
                        lhs = w_sb[:, k, :]
                        for ti, (off, bl) in enumerate(blocks):
                            nc.tensor.matmul(
                                ps[ti][:, :bl],
                                lhs,
                                x_sb[:, k, off : off + bl],
                                start=(k == 0),
                                stop=(k == KT - 1),
                            )
                    for ti, (off, bl) in enumerate(blocks):
                        nc.vector.tensor_tensor(
                            a_sb[:, m, off : off + bl],
                            a_sb[:, m, off : off + bl],
                            ps[ti][:, :bl],
                            mybir.AluOpType.mult,
                        )
                # ---- pass C: y = scale * (Wd^T a) ----
                for m in range(MT):
                    w_sb = wpool.tile([P, KT, P], dt.bfloat16, tag="w")
                    nc.sync.dma_start(w_sb[:], wts[wd_n][:, m])
                    ps = [
                        psum.tile([P, 512], dt.float32, tag=f"ps{ti}", name=f"ps{ti}")
                        for ti in range(len(blocks))
                    ]
                    for k in range(KT):
                        lhs = w_sb[:, k, :]
                        for ti, (off, bl) in enumerate(blocks):
                            nc.tensor.matmul(
                                ps[ti][:, :bl],
                                lhs,
                                a_sb[:, k, off : off + bl],
                                start=(k == 0),
                                stop=(k == KT - 1),
                            )
                    for ti, (off, bl) in enumerate(blocks):
                        y_sb = ypool.tile([P, 512], dt.bfloat16, tag="y")
                        nc.vector.tensor_tensor(
                            y_sb[:, :bl],
                            ps[ti][:, :bl],
                            sc_sb[:, off : off + bl],
                            mybir.AluOpType.mult,
                        )
                        nc.sync.dma_start(y_d[m, :, off : off + bl], y_sb[:, :bl])
    nc.compile()
    return nc


def _get_nc():
    global _NC
    if _NC is None:
        _NC = _build_nc()
    return _NC


def _tile_weight(w):
    """[H(K), I(M)] fp32 -> [P, MT, KT, P] bf16 with out[p,m,k,i] = w[k*P+p, m*P+i]."""
    w = np.asarray(w).astype(BF16)
    return np.ascontiguousarray(w.reshape(KT, P, MT, P).transpose(1, 2, 0, 3))


def _feat_major(x):
    """[T, H] bf16 -> [P, KT, T] with out[p,k,t] = x[t, k*P+p]."""
    T = x.shape[0]
    return np.ascontiguousarray(x.T.reshape(KT, P, T).transpose(1, 0, 2))


def _get_weight_shards(w1, v1, w2, shared_gate, shared_up, shared_down):
    key = (id(w1), id(v1), id(w2), id(shared_gate), id(shared_up), id(shared_down))
    hit = _WEIGHT_CACHE.get(key)
    if hit is not None:
        return hit
    w1 = np.asarray(w1)
    v1 = np.asarray(v1)
    w2 = np.asarray(w2)
    wg_s = _tile_weight(np.asarray(shared_gate).T)  # [I,H] -> [H,I]
    wu_s = _tile_weight(np.asarray(shared_up).T)
    wd_s = _tile_weight(np.asarray(shared_down).T)  # [H,I] -> [I,H]
    shards = []
    for e in range(E):
        shards.append(
            {
                "wg_e": _tile_weight(w1[e]),
                "wu_e": _tile_weight(v1[e]),
                "wd_e": _tile_weight(w2[e]),
                "wg_s": wg_s,
                "wu_s": wu_s,
                "wd_s": wd_s,
            }
        )
    _WEIGHT_CACHE.clear()
    _WEIGHT_CACHE[key] = shards
    return shards


class _Runner:
    """Persistent jit(shard_map(bass_exec)) executable over 8 cores, with
    device-resident weights. Mirrors concourse.bass2jax.run_bass_via_pjrt
    but keeps the traced callable and device buffers alive across calls."""

    def __init__(self, nc):
        import jax
        from jax.sharding import Mesh, NamedSharding, PartitionSpec

        try:
            from jax.experimental.shard_map import shard_map
        except ImportError:
            from jax.shard_map import shard_map
        from concourse.bass2jax import (
            _bass_exec_p,
            install_neuronx_cc_hook,
            partition_id_tensor,
        )

        install_neuronx_cc_hook()
        self.jax = jax
        partition_name = (
            nc.partition_id_tensor.name if nc.partition_id_tensor else None
        )
        in_info = []
        out_names, out_avals = [], []
        for alloc in nc.m.functions[0].allocations:
            if not isinstance(alloc, mybir.MemoryLocationSet):
                continue
            name = alloc.memorylocations[0].name
            if alloc.kind == "ExternalInput":
                if name != partition_name:
                    in_info.append(
                        (name, tuple(alloc.tensor_shape), mybir.dt.np(alloc.dtype))
                    )
            elif alloc.kind == "ExternalOutput":
                out_names.append(name)
                out_avals.append(
                    jax.core.ShapedArray(
                        tuple(alloc.tensor_shape), mybir.dt.np(alloc.dtype)
                    )
                )
        self.in_info = in_info
        self.out_names = out_names
        self.out_avals = out_avals

        in_names = [n for n, _, _ in in_info] + list(out_names)
        if partition_name is not None:
            in_names.append(partition_name)

        def _body(*args):
            operands = list(args)
            if partition_name is not None:
                operands.append(partition_id_tensor())
            outs = _bass_exec_p.bind(
                *operands,
                out_avals=tuple(out_avals),
                in_names=tuple(in_names),
                out_names=tuple(out_names),
                lowering_input_output_aliases=(),
                sim_require_finite=True,
                sim_require_nnan=True,
                nc=nc,
            )
            return tuple(outs)

        mesh = Mesh(np.asarray(jax.devices()[:E]), ("core",))
        spec = PartitionSpec("core")
        n_args = len(in_info) + len(out_names)
        self.fn = jax.jit(
            shard_map(
                _body,
                mesh=mesh,
                in_specs=(spec,) * n_args,
                out_specs=(spec,) * len(out_names),
                check_rep=False,
            ),
            keep_unused=True,
        )
        self.sharding = NamedSharding(mesh, spec)
        # reusable zero output operands (kernel writes every element)
        self.zero_outs = [
            jax.device_put(
                np.zeros((E * av.shape[0], *av.shape[1:]), av.dtype), self.sharding
            )
            for av in out_avals
        ]

    def put(self, per_core_arrays):
        """Concat per-core arrays along axis 0 and place sharded on device."""
        return self.jax.device_put(
            np.concatenate(per_core_arrays, axis=0), self.sharding
        )

    def run(self, bound):
        """bound: dict name -> sharded device array for every input name.
        Returns dict name -> np global array [E*dim0, ...]."""
        args = [bound[name] for name, _, _ in self.in_info]
        outs = self.fn(*args, *self.zero_outs)
        return {n: np.asarray(o) for n, o in zip(self.out_names, outs)}


def _get_runner():
    global _RUNNER
    if _RUNNER is None:
        _RUNNER = _Runner(_get_nc())
    return _RUNNER


def _get_dev_weights(runner, wshards):
    """Device-resident concatenated weight shards, cached across calls."""
    key = tuple(id(wshards[e]["wg_e"]) for e in range(E))
    hit = _DEV_WEIGHTS.get(key)
    if hit is not None:
        return hit
    _DEV_WEIGHTS.clear()
    dev = {
        name: runner.put([wshards[e][name] for e in range(E)])
        for name in WEIGHT_NAMES
    }
    _DEV_WEIGHTS[key] = dev
    return dev


def _run_pass(runner, dev_weights, xbf, wt, idx_lists, add_shared):
    """One SPMD execution over 8 cores. idx_lists[e] is the token-index array
    (<= CE) for expert e this pass. Returns (per-core results list)."""
    acts = {name: [] for name in ACT_NAMES}
    for e in range(E):
        idx = idx_lists[e]
        n = len(idx)
        xe_h = np.zeros((CE, H), dtype=BF16)
        if n:
            xe_h[:n] = xbf[idx]
        sc = np.zeros((CE,), dtype=np.float32)
        if n:
            sc[:n] = wt[idx]
        if add_shared:
            xs_h = xbf[e * CS : (e + 1) * CS]
            scs = np.ones((CS,), dtype=np.float32)
        else:
            xs_h = np.zeros((CS, H), dtype=BF16)
            scs = np.zeros((CS,), dtype=np.float32)
        acts["xe"].append(_feat_major(xe_h))
        acts["xs"].append(_feat_major(xs_h))
        acts["sce"].append(np.ascontiguousarray(np.broadcast_to(sc, (P, CE))))
        acts["scs"].append(np.ascontiguousarray(np.broadcast_to(scs, (P, CS))))
    bound = dict(dev_weights)
    for name in ACT_NAMES:
        bound[name] = runner.put(acts[name])
    glob = runner.run(bound)
    results = []
    for e in range(E):
        results.append(
            {
                n: glob[n].reshape(E, *runner.out_avals[i].shape)[e]
                for i, n in enumerate(runner.out_names)
            }
        )
    return results


def kernel(
    hidden_states,
    router_w,
    w1,
    v1,
    w2,
    shared_gate,
    shared_up,
    shared_down,
):
    hidden_states = np.asarray(hidden_states, dtype=np.float32)
    router_w = np.asarray(router_w, dtype=np.float32)

    B, S, _ = hidden_states.shape
    x = hidden_states.reshape(-1, H)  # [T, H]
    T = x.shape[0]

    # --- routing (host side, part of sharding) ---
    logits = x @ router_w.T  # [T, E]
    top = np.argmax(logits, axis=1)
    wt = 1.0 / (1.0 + np.exp(-logits[np.arange(T), top]))  # sigmoid(top logit)

    runner = _get_runner()
    wshards = _get_weight_shards(w1, v1, w2, shared_gate, shared_up, shared_down)
    dev_weights = _get_dev_weights(runner, wshards)
    xbf = x.astype(BF16)

    per_expert = [np.nonzero(top == e)[0] for e in range(E)]

    out = np.zeros((T, H), dtype=np.float32)
    first = True
    while first or any(len(ix) for ix in per_expert):
        idx_lists = [ix[:CE] for ix in per_expert]
        per_expert = [ix[CE:] for ix in per_expert]
        results = _run_pass(runner, dev_weights, xbf, wt, idx_lists, add_shared=first)
        for e in range(E):
            idx = idx_lists[e]
            n = len(idx)
            if n:
                ye = results[e]["ye"]  # [MT, P, CE] bf16
                y2 = ye.transpose(2, 0, 1).reshape(CE, H)[:n]
                out[idx] += y2.astype(np.float32)
            if first:
                ysh = results[e]["ys"]  # [MT, P, CS] bf16
                out[e * CS : (e + 1) * CS] += (
                    ysh.transpose(2, 0, 1).reshape(CS, H).astype(np.float32)
                )
        first = False

    return out.reshape(B, S, H)



# revision 13
# speedup vs baseline: 1.0171x; 1.0171x over previous
"""Llama4-style MoE (top-1 router + 8 GLU experts + shared GLU expert) on 8
Trainium2 NeuronCores.

Strategy (expert-parallel): the router is evaluated on the host as part of
sharding; tokens are gathered per expert and core e processes expert e's
tokens through its expert GLU, plus a fixed 1/8 shard of all tokens through
the (replicated) shared expert GLU. Matmuls run in bf16 with fp32 PSUM
accumulation. Outputs are scattered back and summed on the host.

Shapes are hardcoded for B=4, S=2048, H=I=2048, E=8.
"""

import sys

for _p in ("/opt/trn_rl_repo", "/root/.axon_site/_ro/trn_rl_repo"):
    if _p not in sys.path:
        sys.path.append(_p)

import numpy as np
import ml_dtypes

import concourse.bass as bass
import concourse.mybir as mybir
import concourse.tile as tile
from concourse import bacc
from concourse.bass_utils import run_bass_kernel_spmd

BF16 = ml_dtypes.bfloat16

P = 128
H = 2048
I = 2048
E = 8
T_TOTAL = 8192
KT = H // P  # 16 k-tiles
MT = I // P  # 16 m-tiles

CE = 1088  # per-core expert-token capacity (mean 1024; actual max for the
# fixed seed is 1078; the multi-pass fallback in kernel() covers overflow)
CS = T_TOTAL // E  # shared-expert tokens per core

EXP_BLOCKS = [(0, 512), (512, 512), (1024, 64)]
SH_BLOCKS = [(0, 512), (512, 512)]

_NC = None  # compiled Bass module (built once per process)
_WEIGHT_CACHE = {}  # id(array) -> preprocessed per-core weight shards
_RUNNER = None  # cached jitted SPMD executable (built once per process)
_DEV_WEIGHTS = {}  # weight id key -> device-resident concatenated shards

WEIGHT_NAMES = ("wg_e", "wu_e", "wd_e", "wg_s", "wu_s", "wd_s")
ACT_NAMES = ("xe", "xs", "sce", "scs")


def _build_nc(reps=1):
    dt = mybir.dt
    nc = bacc.Bacc("TRN2", target_bir_lowering=False, debug=False, num_devices=8)

    xe = nc.dram_tensor("xe", [P, KT, CE], dt.bfloat16, kind="ExternalInput").ap()
    xs = nc.dram_tensor("xs", [P, KT, CS], dt.bfloat16, kind="ExternalInput").ap()
    sce = nc.dram_tensor("sce", [P, CE], dt.float32, kind="ExternalInput").ap()
    scs = nc.dram_tensor("scs", [P, CS], dt.float32, kind="ExternalInput").ap()
    wts = {}
    for name in ("wg_e", "wu_e", "wd_e", "wg_s", "wu_s", "wd_s"):
        wts[name] = nc.dram_tensor(
            name, [P, MT, KT, P], dt.bfloat16, kind="ExternalInput"
        ).ap()
    ye = nc.dram_tensor("ye", [MT, P, CE], dt.bfloat16, kind="ExternalOutput").ap()
    ys = nc.dram_tensor("ys", [MT, P, CS], dt.bfloat16, kind="ExternalOutput").ap()

    with tile.TileContext(nc) as tc:
        with (
            tc.tile_pool(name="xpool", bufs=1) as xpool,
            tc.tile_pool(name="wpool", bufs=6) as wpool,
            tc.tile_pool(name="apool", bufs=1) as apool,
            tc.tile_pool(name="ypool", bufs=4) as ypool,
            tc.tile_pool(name="psum", bufs=2, space="PSUM") as psum,
        ):
            xe_sb = xpool.tile([P, KT, CE], dt.bfloat16, tag="xe")
            xs_sb = xpool.tile([P, KT, CS], dt.bfloat16, tag="xs")
            # startup critical path: first gate-weight tiles and xe[k=0] go
            # out before the bulk x loads so PE can start ~2 us in; per-k
            # xe loads + byte-range dep tracking let pass A's matmuls chase
            # the DMA stream
            pre_w = []
            for m in range(2):
                w_sb = wpool.tile([P, KT, P], dt.bfloat16, tag="w")
                pre_w.append(w_sb)
            nc.sync.dma_start(pre_w[0][:], wts["wg_e"][:, 0])
            nc.sync.dma_start(xe_sb[:, 0], xe[:, 0])
            nc.sync.dma_start(pre_w[1][:], wts["wg_e"][:, 1])
            for k in range(1, KT):
                nc.sync.dma_start(xe_sb[:, k], xe[:, k])
            sce_sb = xpool.tile([P, CE], dt.float32, tag="sce")
            scs_sb = xpool.tile([P, CS], dt.float32, tag="scs")
            ae_sb = apool.tile([P, MT, CE], dt.bfloat16, tag="ae")
            as_sb = apool.tile([P, MT, CS], dt.bfloat16, tag="as")

            groups = [
                (xe_sb, sce_sb, ae_sb, "wg_e", "wu_e", "wd_e", ye, EXP_BLOCKS),
                (xs_sb, scs_sb, as_sb, "wg_s", "wu_s", "wd_s", ys, SH_BLOCKS),
            ] * reps
            for gi, (x_sb, sc_sb, a_sb, wg_n, wu_n, wd_n, y_d, blocks) in enumerate(
                groups
            ):
                # ---- pass A: a = silu(Wg^T x) ----
                for m in range(MT):
                    if gi == 0 and m < 2:
                        w_sb = pre_w[m]
                    else:
                        w_sb = wpool.tile([P, KT, P], dt.bfloat16, tag="w")
                        nc.sync.dma_start(w_sb[:], wts[wg_n][:, m])
                    ps = [
                        psum.tile([P, 512], dt.float32, tag=f"ps{ti}", name=f"ps{ti}")
                        for ti in range(len(blocks))
                    ]
                    for k in range(KT):
                        lhs = w_sb[:, k, :]
                        for ti, (off, bl) in enumerate(blocks):
                            nc.tensor.matmul(
                                ps[ti][:, :bl],
                                lhs,
                                x_sb[:, k, off : off + bl],
                                start=(k == 0),
                                stop=(k == KT - 1),
                            )
                    for ti, (off, bl) in enumerate(blocks):
                        nc.scalar.activation(
                            a_sb[:, m, off : off + bl],
                            ps[ti][:, :bl],
                            mybir.ActivationFunctionType.Silu,
                        )
                if gi == 0:
                    # xs/sce/scs are first read in expert pass C and the
                    # shared group; issuing after pass A keeps them off the
                    # startup DMA critical path
                    for k in range(KT):
                        nc.sync.dma_start(xs_sb[:, k], xs[:, k])
                    nc.sync.dma_start(sce_sb[:], sce[:])
                    nc.sync.dma_start(scs_sb[:], scs[:])
                # ---- pass B: a *= Wu^T x ----
                for m in range(MT):
                    w_sb = wpool.tile([P, KT, P], dt.bfloat16, tag="w")
                    nc.sync.dma_start(w_sb[:], wts[wu_n][:, m])
                    ps = [
                        psum.tile([P, 512], dt.float32, tag=f"ps{ti}", name=f"ps{ti}")
                        for ti in range(len(blocks))
                    ]
                    for k in range(KT):
                        lhs = w_sb[:, k, :]
                        for ti, (off, bl) in enumerate(blocks):
                            nc.tensor.matmul(
                                ps[ti][:, :bl],
                                lhs,
                                x_sb[:, k, off : off + bl],
                                start=(k == 0),
                                stop=(k == KT - 1),
                            )
                    for ti, (off, bl) in enumerate(blocks):
                        nc.vector.tensor_tensor(
                            a_sb[:, m, off : off + bl],
                            a_sb[:, m, off : off + bl],
                            ps[ti][:, :bl],
                            mybir.AluOpType.mult,
                        )
                # ---- pass C: y = scale * (Wd^T a) ----
                for m in range(MT):
                    w_sb = wpool.tile([P, KT, P], dt.bfloat16, tag="w")
                    nc.sync.dma_start(w_sb[:], wts[wd_n][:, m])
                    ps = [
                        psum.tile([P, 512], dt.float32, tag=f"ps{ti}", name=f"ps{ti}")
                        for ti in range(len(blocks))
                    ]
                    for k in range(KT):
                        lhs = w_sb[:, k, :]
                        for ti, (off, bl) in enumerate(blocks):
                            nc.tensor.matmul(
                                ps[ti][:, :bl],
                                lhs,
                                a_sb[:, k, off : off + bl],
                                start=(k == 0),
                                stop=(k == KT - 1),
                            )
                    for ti, (off, bl) in enumerate(blocks):
                        y_sb = ypool.tile([P, 512], dt.bfloat16, tag="y")
                        nc.vector.tensor_tensor(
                            y_sb[:, :bl],
                            ps[ti][:, :bl],
                            sc_sb[:, off : off + bl],
                            mybir.AluOpType.mult,
                        )
                        nc.sync.dma_start(y_d[m, :, off : off + bl], y_sb[:, :bl])
    nc.compile()
    return nc


def _get_nc():
    global _NC
    if _NC is None:
        _NC = _build_nc()
    return _NC


def _tile_weight(w):
    """[H(K), I(M)] fp32 -> [P, MT, KT, P] bf16 with out[p,m,k,i] = w[k*P+p, m*P+i]."""
    w = np.asarray(w).astype(BF16)
    return np.ascontiguousarray(w.reshape(KT, P, MT, P).transpose(1, 2, 0, 3))


def _feat_major(x):
    """[T, H] bf16 -> [P, KT, T] with out[p,k,t] = x[t, k*P+p]."""
    T = x.shape[0]
    return np.ascontiguousarray(x.T.reshape(KT, P, T).transpose(1, 0, 2))


def _get_weight_shards(w1, v1, w2, shared_gate, shared_up, shared_down):
    key = (id(w1), id(v1), id(w2), id(shared_gate), id(shared_up), id(shared_down))
    hit = _WEIGHT_CACHE.get(key)
    if hit is not None:
        return hit
    w1 = np.asarray(w1)
    v1 = np.asarray(v1)
    w2 = np.asarray(w2)
    wg_s = _tile_weight(np.asarray(shared_gate).T)  # [I,H] -> [H,I]
    wu_s = _tile_weight(np.asarray(shared_up).T)
    wd_s = _tile_weight(np.asarray(shared_down).T)  # [H,I] -> [I,H]
    shards = []
    for e in range(E):
        shards.append(
            {
                "wg_e": _tile_weight(w1[e]),
                "wu_e": _tile_weight(v1[e]),
                "wd_e": _tile_weight(w2[e]),
                "wg_s": wg_s,
                "wu_s": wu_s,
                "wd_s": wd_s,
            }
        )
    _WEIGHT_CACHE.clear()
    _WEIGHT_CACHE[key] = shards
    return shards


class _Runner:
    """Persistent jit(shard_map(bass_exec)) executable over 8 cores, with
    device-resident weights. Mirrors concourse.bass2jax.run_bass_via_pjrt
    but keeps the traced callable and device buffers alive across calls."""

    def __init__(self, nc):
        import jax
        from jax.sharding import Mesh, NamedSharding, PartitionSpec

        try:
            from jax.experimental.shard_map import shard_map
        except ImportError:
            from jax.shard_map import shard_map
        from concourse.bass2jax import (
            _bass_exec_p,
            install_neuronx_cc_hook,
            partition_id_tensor,
        )

        install_neuronx_cc_hook()
        self.jax = jax
        partition_name = (
            nc.partition_id_tensor.name if nc.partition_id_tensor else None
        )
        in_info = []
        out_names, out_avals = [], []
        for alloc in nc.m.functions[0].allocations:
            if not isinstance(alloc, mybir.MemoryLocationSet):
                continue
            name = alloc.memorylocations[0].name
            if alloc.kind == "ExternalInput":
                if name != partition_name:
                    in_info.append(
                        (name, tuple(alloc.tensor_shape), mybir.dt.np(alloc.dtype))
                    )
            elif alloc.kind == "ExternalOutput":
                out_names.append(name)
                out_avals.append(
                    jax.core.ShapedArray(
                        tuple(alloc.tensor_shape), mybir.dt.np(alloc.dtype)
                    )
                )
        self.in_info = in_info
        self.out_names = out_names
        self.out_avals = out_avals

        in_names = [n for n, _, _ in in_info] + list(out_names)
        if partition_name is not None:
            in_names.append(partition_name)

        def _body(*args):
            operands = list(args)
            if partition_name is not None:
                operands.append(partition_id_tensor())
            outs = _bass_exec_p.bind(
                *operands,
                out_avals=tuple(out_avals),
                in_names=tuple(in_names),
                out_names=tuple(out_names),
                lowering_input_output_aliases=(),
                sim_require_finite=True,
                sim_require_nnan=True,
                nc=nc,
            )
            return tuple(outs)

        mesh = Mesh(np.asarray(jax.devices()[:E]), ("core",))
        spec = PartitionSpec("core")
        n_args = len(in_info) + len(out_names)
        self.fn = jax.jit(
            shard_map(
                _body,
                mesh=mesh,
                in_specs=(spec,) * n_args,
                out_specs=(spec,) * len(out_names),
                check_rep=False,
            ),
            keep_unused=True,
        )
        self.sharding = NamedSharding(mesh, spec)
        # reusable zero output operands (kernel writes every element)
        self.zero_outs = [
            jax.device_put(
                np.zeros((E * av.shape[0], *av.shape[1:]), av.dtype), self.sharding
            )
            for av in out_avals
        ]

    def put(self, per_core_arrays):
        """Concat per-core arrays along axis 0 and place sharded on device."""
        return self.jax.device_put(
            np.concatenate(per_core_arrays, axis=0), self.sharding
        )

    def run(self, bound):
        """bound: dict name -> sharded device array for every input name.
        Returns dict name -> np global array [E*dim0, ...]."""
        args = [bound[name] for name, _, _ in self.in_info]
        outs = self.fn(*args, *self.zero_outs)
        return {n: np.asarray(o) for n, o in zip(self.out_names, outs)}


def _get_runner():
    global _RUNNER
    if _RUNNER is None:
        _RUNNER = _Runner(_get_nc())
    return _RUNNER


def _get_dev_weights(runner, wshards):
    """Device-resident concatenated weight shards, cached across calls."""
    key = tuple(id(wshards[e]["wg_e"]) for e in range(E))
    hit = _DEV_WEIGHTS.get(key)
    if hit is not None:
        return hit
    _DEV_WEIGHTS.clear()
    dev = {
        name: runner.put([wshards[e][name] for e in range(E)])
        for name in WEIGHT_NAMES
    }
    _DEV_WEIGHTS[key] = dev
    return dev


def _run_pass(runner, dev_weights, xbf, wt, idx_lists, add_shared):
    """One SPMD execution over 8 cores. idx_lists[e] is the token-index array
    (<= CE) for expert e this pass. Returns (per-core results list)."""
    acts = {name: [] for name in ACT_NAMES}
    for e in range(E):
        idx = idx_lists[e]
        n = len(idx)
        xe_h = np.zeros((CE, H), dtype=BF16)
        if n:
            xe_h[:n] = xbf[idx]
        sc = np.zeros((CE,), dtype=np.float32)
        if n:
            sc[:n] = wt[idx]
        if add_shared:
            xs_h = xbf[e * CS : (e + 1) * CS]
            scs = np.ones((CS,), dtype=np.float32)
        else:
            xs_h = np.zeros((CS, H), dtype=BF16)
            scs = np.zeros((CS,), dtype=np.float32)
        acts["xe"].append(_feat_major(xe_h))
        acts["xs"].append(_feat_major(xs_h))
        acts["sce"].append(np.ascontiguousarray(np.broadcast_to(sc, (P, CE))))
        acts["scs"].append(np.ascontiguousarray(np.broadcast_to(scs, (P, CS))))
    bound = dict(dev_weights)
    for name in ACT_NAMES:
        bound[name] = runner.put(acts[name])
    glob = runner.run(bound)
    results = []
    for e in range(E):
        results.append(
            {
                n: glob[n].reshape(E, *runner.out_avals[i].shape)[e]
                for i, n in enumerate(runner.out_names)
            }
        )
    return results


def kernel(
    hidden_states,
    router_w,
    w1,
    v1,
    w2,
    shared_gate,
    shared_up,
    shared_down,
):
    hidden_states = np.asarray(hidden_states, dtype=np.float32)
    router_w = np.asarray(router_w, dtype=np.float32)

    B, S, _ = hidden_states.shape
    x = hidden_states.reshape(-1, H)  # [T, H]
    T = x.shape[0]

    # --- routing (host side, part of sharding) ---
    logits = x @ router_w.T  # [T, E]
    top = np.argmax(logits, axis=1)
    wt = 1.0 / (1.0 + np.exp(-logits[np.arange(T), top]))  # sigmoid(top logit)

    runner = _get_runner()
    wshards = _get_weight_shards(w1, v1, w2, shared_gate, shared_up, shared_down)
    dev_weights = _get_dev_weights(runner, wshards)
    xbf = x.astype(BF16)

    per_expert = [np.nonzero(top == e)[0] for e in range(E)]

    out = np.zeros((T, H), dtype=np.float32)
    first = True
    while first or any(len(ix) for ix in per_expert):
        idx_lists = [ix[:CE] for ix in per_expert]
        per_expert = [ix[CE:] for ix in per_expert]
        results = _run_pass(runner, dev_weights, xbf, wt, idx_lists, add_shared=first)
        for e in range(E):
            idx = idx_lists[e]
            n = len(idx)
            if n:
                ye = results[e]["ye"]  # [MT, P, CE] bf16
                y2 = ye.transpose(2, 0, 1).reshape(CE, H)[:n]
                out[idx] += y2.astype(np.float32)
            if first:
                ysh = results[e]["ys"]  # [MT, P, CS] bf16
                out[e * CS : (e + 1) * CS] += (
                    ysh.transpose(2, 0, 1).reshape(CS, H).astype(np.float32)
                )
        first = False

    return out.reshape(B, S, H)

